# revision 27
# baseline (speedup 1.0000x reference)
"""Tensor-parallel MultiHeadAttention (QKV + RoPE + GQA causal SDPA + dense)
for 8 Trainium2 NeuronCores.

Sharding (TP as in TPMultiHeadAttention): core d owns query heads {2d, 2d+1}
and the single kv head d//2 (kv heads replicated across core pairs), plus the
matching 256 columns of the dense projection. Each core produces a full-shape
partial output; the all-reduce is a host-side sum over the 8 partials.

v2: full bf16 pipeline (PE streams bf16 at the same 1 col/cycle as float32r,
but DMA bytes, DVE element ops, and LDWEIGHTS (FWL) all halve).

Per-core device pipeline:
  1. qkv^T = W_shard @ x^T            -> [f=512, s=2048] (f on partitions)
  2. RoPE on q,k via a permutation matmul (rotate_half) + DVE combine;
     softmax scale folded into the q-side weights
  3. Attention per 512-query chunk, with score k-tiles processed in PAIRS:
     two N=512 score matmuls write bf16 into one shared PSUM bank
     ([128,1024] bf16), one ScalarE exp evacuates the pair, diagonal pairs
     are masked multiplicatively with a host mask constant (diagonal tiles
     are computed full-width so the pair layout stays uniform), the pair is
     added into a bf16 running accumulator (softmax denominator), and two
     v-matmuls accumulate ctx^T in fp32 PSUM.
  4. Denominators: rp[128,512] = allones128.T @ acc via two PE matmuls
     (the sum lands REPLICATED across all 128 partitions - no gpsimd
     broadcast), reciprocal'd on DVE, multiplied into ctx^T.
  5. out[s, e] += ctx^T.T @ wd^T  (accumulate over the 2 local heads),
     written out as bf16; host sums the 8 partials in fp32.
"""

import numpy as np
import ml_dtypes

BF16 = ml_dtypes.bfloat16

B, S, E = 1, 2048, 2048
H, KVH, D = 16, 4, 128
NCORES = 8
P = 128
FD = 512            # matmul moving free dim == one fp32 PSUM bank
NE = E // P         # 16 contraction tiles over the embedding dim
NSC = S // FD       # 4 sequence chunks
NST = S // P        # 16 sequence tiles
FLOC = 4 * P        # local fused qkv rows per core (2 q heads + k + v)
ROPE_BASE = 10000.0

LAST_RESULT = None
_BASS_CACHE = None


def _rope_tables():
    inv = 1.0 / (ROPE_BASE ** (np.arange(0, D, 2, dtype=np.float64) / D))
    t = np.arange(S, dtype=np.float64)
    freqs = np.outer(t, inv)
    emb = np.concatenate([freqs, freqs], axis=-1)  # [S, D]
    return np.cos(emb), np.sin(emb)


def _diag_masks():
    # mask for diagonal pair g (tiles o=2g, 2g+1), laid out [P, 2*FD]:
    # tile o occupies cols [512*(o%2), 512*(o%2)+512); element (r, q) of
    # tile o is visible iff q >= 128*o + r  (q, r local to the 512-chunk)
    q = np.arange(FD)[None, :]
    r = np.arange(P)[:, None]
    masks = []
    for g in range(2):
        cols = []
        for o in (2 * g, 2 * g + 1):
            cols.append((q >= 128 * o + r).astype(np.float32))
        masks.append(np.concatenate(cols, axis=1))
    return masks  # two [P, 1024] arrays


def _host_constants():
    cos, sin = _rope_tables()
    m0, m1 = _diag_masks()
    consts = {
        "cosr": np.ascontiguousarray(cos.T).astype(BF16),
        "sinr": np.ascontiguousarray(sin.T).astype(BF16),
        "mk0": np.ascontiguousarray(m0).astype(BF16),
        "mk1": np.ascontiguousarray(m1).astype(BF16),
        "onm": np.ones((P, P), np.float32).astype(BF16),
        "ident": np.eye(P, dtype=np.float32).astype(BF16),
    }
    # rotate_half as a matmul: rot = M @ q (in [d, s] layout); pass M.T as lhsT
    M = np.zeros((P, P), np.float32)
    half = D // 2
    M[np.arange(half), np.arange(half) + half] = -1.0
    M[np.arange(half) + half, np.arange(half)] = 1.0
    consts["protT"] = np.ascontiguousarray(M.T).astype(BF16)
    return consts


def _build_bass():
    import concourse.mybir as mybir
    import concourse.tile as tile
    from concourse import bacc

    f32 = mybir.dt.float32
    bf16 = mybir.dt.bfloat16
    Exp = mybir.ActivationFunctionType.Exp

    nc = bacc.Bacc(None, target_bir_lowering=False, name="mha_tp8")
    # x pre-tiled on host to [sc, g, p, i, f] (4 eo-tiles per 512KB DMA) so
    # every transfer is large and fully contiguous (DMA issue cost on the
    # sync queue engine is ~600ns per dma_start regardless of size)
    xTt = nc.dram_tensor("xTt", [NSC, 4, P, 4, FD], bf16, kind="ExternalInput")
    wqkvT = nc.dram_tensor("wqkvT", [4, P, 4, FLOC], bf16, kind="ExternalInput")
    wdT = nc.dram_tensor("wdT", [2 * P, S], bf16, kind="ExternalInput")
    cosr = nc.dram_tensor("cosr", [P, S], bf16, kind="ExternalInput")
    sinr = nc.dram_tensor("sinr", [P, S], bf16, kind="ExternalInput")
    mk0d = nc.dram_tensor("mk0", [P, 2 * FD], bf16, kind="ExternalInput")
    mk1d = nc.dram_tensor("mk1", [P, 2 * FD], bf16, kind="ExternalInput")
    protT = nc.dram_tensor("protT", [P, P], bf16, kind="ExternalInput")
    ident = nc.dram_tensor("ident", [P, P], bf16, kind="ExternalInput")
    onmd = nc.dram_tensor("onm", [P, P], bf16, kind="ExternalInput")
    # output tiled [c, st, ep, p, k, f] (eo-pairs per 256KB store); host
    # reassembles to [s, e]
    out = nc.dram_tensor("out", [NSC, 4, 2, P, 2, FD], bf16, kind="ExternalOutput")

    with tile.TileContext(nc) as tc:
        with tc.tile_pool(name="const", bufs=1) as const:
            w_sb = const.tile([P, NE, FLOC], bf16, name="w_sb")
            pr = const.tile([P, P], bf16, name="pr")
            idn = const.tile([P, P], bf16, name="idn")
            onm = const.tile([P, P], bf16, name="onm")

            cq = const.tile([P, S], bf16, name="cq")
            sq_t = const.tile([P, S], bf16, name="sq_t")
            mk = [
                const.tile([P, 2 * FD], bf16, name="mk0"),
                const.tile([P, 2 * FD], bf16, name="mk1"),
            ]
            wd_sb = const.tile([P, 2, S], bf16, name="wd_sb")

            qr = const.tile([P, 2, S], bf16, name="qr")
            kr = const.tile([P, S], bf16, name="kr")
            vT = const.tile([P, S], bf16, name="vT")
            vn = const.tile([P, NST, P], bf16, name="vn")

            # ---- Phase A: fused QKV projection + RoPE + v transpose ----
            with tc.tile_pool(name="xs_p", bufs=6) as xpool, \
                 tc.tile_pool(name="ps_qkv", bufs=1, space="PSUM") as pqkv, \
                 tc.tile_pool(name="ps_rot", bufs=2, space="PSUM") as prot_p, \
                 tc.tile_pool(name="ps_vt", bufs=2, space="PSUM") as pvt, \
                 tc.tile_pool(name="rtmp", bufs=3) as rtmp:
                # x tiles alternate between the sync and scalar DMA rings so
                # the first chunks stream in at 2x rate; weights interleave on
                # the sync ring; tables/masks/dense weights load once the
                # first chunk's tiles are queued
                for sc in range(NSC):
                    ssl = slice(sc * FD, (sc + 1) * FD)
                    psums = [
                        pqkv.tile([P, FD], f32, tag=f"qkv{f}", name=f"ps_qkv{f}_{sc}")
                        for f in range(4)
                    ]
                    if sc == 1:
                        nc.gpsimd.dma_start(wd_sb, wdT.rearrange("(h p) e -> p h e", p=P))
                        nc.gpsimd.dma_start(mk[0], mk0d[:, :])
                        nc.gpsimd.dma_start(mk[1], mk1d[:, :])
                    for g in range(4):
                        # weights and x tiles pair up across the sync (HWDGE)
                        # and idle gpsimd (SWDGE) rings: two DMA streams in
                        # parallel, and the scalar engine keeps its queue free
                        # for the psum-evacuating copies
                        ring = nc.sync if g % 2 == 0 else nc.gpsimd
                        xs = xpool.tile(
                            [P, 4, FD], bf16, tag="xs", name=f"xs_{sc}_{g}"
                        )
                        if sc == 0 and g < 2:
                            # starter split: the first eo-slice of each ring
                            # lands alone so the first matmuls start ~5us
                            # earlier (all 8 cores contend for HBM here)
                            ring.dma_start(w_sb[:, 4 * g, :], wqkvT[g][:, 0, :])
                            ring.dma_start(xs[:, 0, :], xTt[sc, g][:, 0, :])
                            ring.dma_start(
                                w_sb[:, 4 * g + 1:4 * g + 4, :], wqkvT[g][:, 1:, :]
                            )
                            ring.dma_start(xs[:, 1:, :], xTt[sc, g][:, 1:, :])
                        else:
                            if sc == 0:
                                ring.dma_start(w_sb[:, 4 * g:4 * g + 4, :], wqkvT[g])
                            ring.dma_start(xs, xTt[sc, g])
                        if sc == 0 and g == 2:
                            nc.gpsimd.dma_start(cq, cosr[:, :])
                            nc.gpsimd.dma_start(sq_t, sinr[:, :])
                        if sc == 0 and g == 3:
                            nc.sync.dma_start(pr, protT[:, :])
                            nc.sync.dma_start(idn, ident[:, :])
                            nc.sync.dma_start(onm, onmd[:, :])
                        for i in range(4):
                            eo = 4 * g + i
                            for f in range(4):
                                nc.tensor.matmul(
                                    psums[f],
                                    lhsT=w_sb[:, eo, f * P:(f + 1) * P],
                                    rhs=xs[:, i, :],
                                    start=(eo == 0),
                                    stop=(eo == NE - 1),
                                )
                    # psum-freeing copies on ScalarE (idle in this phase)
                    for f in range(4):
                        pt = psums[f]
                        if f == 3:
                            nc.scalar.copy(vT[:, ssl], pt)
                            continue
                        dst = qr[:, f, ssl] if f < 2 else kr[:, ssl]
                        qt = rtmp.tile([P, FD], bf16, tag="qt", name=f"qt_{sc}_{f}")
                        nc.scalar.copy(qt, pt)
                        rp = prot_p.tile([P, FD], f32, tag="rot", name=f"rot_{sc}_{f}")
                        nc.tensor.matmul(rp, lhsT=pr, rhs=qt, start=True, stop=True)
                        tt = rtmp.tile([P, FD], bf16, tag="tt", name=f"tt_{sc}_{f}")
                        nc.vector.tensor_mul(tt, rp, sq_t[:, ssl])
                        nc.vector.tensor_mul(dst, qt, cq[:, ssl])
                        nc.vector.tensor_add(dst, dst, tt)
                    # v transpose: 4 PE transposes packed into one PSUM bank,
                    # one ScalarE copy evacuates all four
                    vp = pvt.tile([P, 4 * P], bf16, tag="vt", name=f"vt_{sc}")
                    for k in range(4):
                        j = 4 * sc + k
                        nc.tensor.transpose(
                            vp[:, k * P:(k + 1) * P], vT[:, j * P:(j + 1) * P], idn
                        )
                    nc.scalar.copy(vn[:, 4 * sc:4 * sc + 4, :], vp)

            # ---- Phase B: attention + dense, per 512-query chunk ----
            # Emission order interleaves dense(c) after attention(c+1) so the
            # PE always has independent work during each chunk's softmax tail.
            with tc.tile_pool(name="ps_s", bufs=2, space="PSUM") as ps_s, \
                 tc.tile_pool(name="ps_ctx", bufs=2, space="PSUM") as ps_ctx, \
                 tc.tile_pool(name="ps_rp", bufs=1, space="PSUM") as ps_rp, \
                 tc.tile_pool(name="ps_o", bufs=3, space="PSUM") as ps_o, \
                 tc.tile_pool(name="pt_p", bufs=3) as ptp, \
                 tc.tile_pool(name="acc_p", bufs=3) as accp, \
                 tc.tile_pool(name="rec_p", bufs=2) as recp, \
                 tc.tile_pool(name="ctx_p", bufs=2) as ctxp, \
                 tc.tile_pool(name="out_p", bufs=4) as outp:
                all_csb = {}
                # dense work is emitted as "filler" units (one eo-pair: 4
                # matmuls + 2 evacs + 1 store) sprinkled into the NEXT
                # chunk's attention stream, so dense matmuls cover the PE
                # bubbles left by exp/acc/recip dependency chains
                fillers = []

                def pop_filler(n=1):
                    for _ in range(n):
                        if fillers:
                            fillers.pop(0)()

                def emit_attn(c):
                    qbase = c * FD
                    npairs = 2 * c + 2
                    ctxps, accs = {}, {}
                    for h in range(2):
                        ctxps[h] = ps_ctx.tile(
                            [P, FD], f32, tag="ctx", name=f"ctx_{c}_{h}"
                        )
                        acc = accp.tile(
                            [P, 2 * FD], bf16, tag="acc", name=f"acc_{c}_{h}"
                        )
                        accs[h] = acc
                        qsl = qr[:, h, qbase:qbase + FD]
                        for pi in range(npairs):
                            pt = ptp.tile(
                                [P, 2 * FD], bf16, tag="pt", name=f"pt_{c}_{h}_{pi}"
                            )
                            for half in range(2):
                                j = 2 * pi + half
                                sp = ps_s.tile(
                                    [P, FD], f32, tag="s", name=f"s_{c}_{h}_{j}"
                                )
                                nc.tensor.matmul(
                                    sp,
                                    lhsT=kr[:, j * P:(j + 1) * P],
                                    rhs=qsl, start=True, stop=True,
                                )
                                nc.scalar.activation(
                                    pt[:, half * FD:(half + 1) * FD], sp, Exp
                                )
                            if pi >= npairs - 2:
                                # diagonal pair: zero the causally invisible
                                # region (tiles were computed full-width)
                                nc.vector.tensor_mul(
                                    pt, pt, mk[pi - (npairs - 2)]
                                )
                            if pi == 0:
                                nc.vector.tensor_copy(acc, pt)
                            else:
                                nc.vector.tensor_add(acc, acc, pt)
                            nc.tensor.matmul(
                                ctxps[h],
                                lhsT=vn[:, 2 * pi, :],
                                rhs=pt[:, :FD],
                                start=(pi == 0), stop=False,
                            )
                            nc.tensor.matmul(
                                ctxps[h],
                                lhsT=vn[:, 2 * pi + 1, :],
                                rhs=pt[:, FD:],
                                start=False, stop=(pi == npairs - 1),
                            )

                    # softmax tails after both heads' tile loops; the ones
                    # matmul replicates the denominator across all partitions.
                    # Dense fillers are sandwiched between the two heads'
                    # tails so the PE has work while recip/ct drain on DVE
                    # and the rp bank frees up.
                    for h in range(2):
                        acc = accs[h]
                        rpp = ps_rp.tile([P, FD], f32, tag="rp", name=f"rp_{c}_{h}")
                        nc.tensor.matmul(rpp, lhsT=onm, rhs=acc[:, :FD],
                                         start=True, stop=False)
                        nc.tensor.matmul(rpp, lhsT=onm, rhs=acc[:, FD:],
                                         start=False, stop=True)
                        rec = recp.tile([P, FD], f32, tag="rec", name=f"rec_{c}_{h}")
                        nc.vector.reciprocal_approx_fast(rec, rpp)
                        ct = ctxp.tile([P, FD], bf16, tag=f"ctx{h}", name=f"csb_{c}_{h}")
                        nc.vector.tensor_mul(ct, ctxps[h], rec)
                        all_csb[(c, h)] = ct
                        pop_filler(2)

                def queue_dense(c):
                    def unit(st, ep):
                        def emit():
                            ot = outp.tile(
                                [P, 2, FD], bf16, tag="ot", name=f"ot_{c}_{st}_{ep}"
                            )
                            for k in range(2):
                                eo = 2 * ep + k
                                op = ps_o.tile(
                                    [P, FD], f32, tag="o", name=f"o_{c}_{st}_{eo}"
                                )
                                for h in range(2):
                                    nc.tensor.matmul(
                                        op,
                                        lhsT=all_csb[(c, h)][:, st * P:(st + 1) * P],
                                        rhs=wd_sb[:, h, eo * FD:(eo + 1) * FD],
                                        start=(h == 0), stop=(h == 1),
                                    )
                                if (st + eo) % 2:
                                    nc.scalar.copy(ot[:, k, :], op)
                                else:
                                    nc.vector.tensor_copy(ot[:, k, :], op)
                            nc.sync.dma_start(out[c, st, ep], ot)
                        return emit
                    for st in range(4):
                        for ep in range(2):
                            fillers.append(unit(st, ep))

                # dense(c) runs as a block between attn(c+1) and attn(c+2),
                # except for 4 units sandwiched into attn(c+1)'s softmax
                # tails (covering the rp-bank/recip serialization there)
                emit_attn(0)
                queue_dense(0)
                emit_attn(1)
                pop_filler(len(fillers))
                queue_dense(1)
                emit_attn(2)
                pop_filler(len(fillers))
                queue_dense(2)
                emit_attn(3)
                pop_filler(len(fillers))
                queue_dense(3)
                pop_filler(len(fillers))
    nc.compile()
    return nc


def make_in_maps(x, w_qkv, w_dense):
    x = np.asarray(x, np.float32).reshape(S, E)
    w_qkv = np.asarray(w_qkv, np.float32)
    w_dense = np.asarray(w_dense, np.float32)
    # x^T tiled to [sc, g, p, i, f] (4 eo-tiles per DMA) so device DMAs are
    # large and contiguous
    xTt = np.ascontiguousarray(
        x.T.reshape(4, 4, P, NSC, FD).transpose(3, 0, 2, 1, 4)
    ).astype(BF16)
    consts = _host_constants()
    in_maps = []
    scale = np.float32(1.0 / np.sqrt(D))
    for d in range(NCORES):
        g = d // 2
        wq = w_qkv[2 * d * P:(2 * d + 2) * P] * scale
        wk = w_qkv[H * D + g * P: H * D + (g + 1) * P]
        wv = w_qkv[H * D + KVH * D + g * P: H * D + KVH * D + (g + 1) * P]
        wqkvT_d = np.ascontiguousarray(
            np.concatenate([wq, wk, wv], 0).T.reshape(4, 4, P, FLOC)
            .transpose(0, 2, 1, 3)
        ).astype(BF16)
        wdT_d = np.ascontiguousarray(
            w_dense[:, 2 * d * P:(2 * d + 2) * P].T
        ).astype(BF16)
        m = {"xTt": xTt, "wqkvT": wqkvT_d, "wdT": wdT_d}
        m.update(consts)
        in_maps.append(m)
    return in_maps


def kernel(x, w_qkv, w_dense):
    global LAST_RESULT, _BASS_CACHE
    from concourse.bass_utils import run_bass_kernel_spmd

    in_maps = make_in_maps(x, w_qkv, w_dense)
    if _BASS_CACHE is None:
        _BASS_CACHE = _build_bass()
    res = run_bass_kernel_spmd(_BASS_CACHE, in_maps, core_ids=list(range(NCORES)))
    LAST_RESULT = res
    # sum partials over cores, then untile [c, st, ep, p, k, f] -> [s, e]
    acc = np.zeros((NSC, 4, 2, P, 2, FD), np.float32)
    for r in res.results:
        acc += np.asarray(r["out"], dtype=np.float32)
    # [c, st, ep, p, k, f]: s = (c, st, p), e = (ep, k, f)
    full = acc.transpose(0, 1, 3, 2, 4, 5).reshape(S, E)
    return np.ascontiguousarray(full).reshape(B, S, E)


# revision 31
# speedup vs baseline: 1.0187x; 1.0187x over previous
"""Tensor-parallel MultiHeadAttention (QKV + RoPE + GQA causal SDPA + dense)
for 8 Trainium2 NeuronCores.

Sharding (TP as in TPMultiHeadAttention): core d owns query heads {2d, 2d+1}
and the single kv head d//2 (kv heads replicated across core pairs), plus the
matching 256 columns of the dense projection. Each core produces a full-shape
partial output; the all-reduce is a host-side sum over the 8 partials.

Full bf16 pipeline: the PE streams bf16 at the same 1 col/cycle as float32r
(this problem's previous format), but DMA bytes, DVE element ops, and
LDWEIGHTS (FWL) all halve; rel-err vs the fp32 reference is ~7e-3.

Per-core device pipeline:
  1. qkv^T = W_shard @ x^T            -> [f=512, s=2048] (f on partitions)
  2. RoPE on q,k via a permutation matmul (rotate_half) + DVE combine;
     softmax scale folded into the q-side weights
  3. Attention per 512-query chunk, with score k-tiles processed in PAIRS:
     two N=512 score matmuls write bf16 into one shared PSUM bank
     ([128,1024] bf16), one ScalarE exp evacuates the pair, diagonal pairs
     are masked multiplicatively with a host mask constant (diagonal tiles
     are computed full-width so the pair layout stays uniform), the pair is
     added into a bf16 running accumulator (softmax denominator), and two
     v-matmuls accumulate ctx^T in fp32 PSUM.
  4. Denominators: rp[128,512] = allones128.T @ acc via two PE matmuls
     (the sum lands REPLICATED across all 128 partitions - no gpsimd
     broadcast), reciprocal'd on DVE, multiplied into ctx^T.
  5. out[s, e] += ctx^T.T @ wd^T  (accumulate over the 2 local heads),
     written out as bf16; host sums the 8 partials in fp32.
"""

import numpy as np
import ml_dtypes

BF16 = ml_dtypes.bfloat16

B, S, E = 1, 2048, 2048
H, KVH, D = 16, 4, 128
NCORES = 8
P = 128
FD = 512            # matmul moving free dim == one fp32 PSUM bank
NE = E // P         # 16 contraction tiles over the embedding dim
NSC = S // FD       # 4 sequence chunks
NST = S // P        # 16 sequence tiles
FLOC = 4 * P        # local fused qkv rows per core (2 q heads + k + v)
ROPE_BASE = 10000.0

LAST_RESULT = None
_BASS_CACHE = None


def _rope_tables():
    inv = 1.0 / (ROPE_BASE ** (np.arange(0, D, 2, dtype=np.float64) / D))
    t = np.arange(S, dtype=np.float64)
    freqs = np.outer(t, inv)
    emb = np.concatenate([freqs, freqs], axis=-1)  # [S, D]
    return np.cos(emb), np.sin(emb)


def _diag_masks():
    # mask for diagonal pair g (tiles o=2g, 2g+1), laid out [P, 2*FD]:
    # tile o occupies cols [512*(o%2), 512*(o%2)+512); element (r, q) of
    # tile o is visible iff q >= 128*o + r  (q, r local to the 512-chunk)
    q = np.arange(FD)[None, :]
    r = np.arange(P)[:, None]
    masks = []
    for g in range(2):
        cols = []
        for o in (2 * g, 2 * g + 1):
            cols.append((q >= 128 * o + r).astype(np.float32))
        masks.append(np.concatenate(cols, axis=1))
    return masks  # two [P, 1024] arrays


def _host_constants():
    cos, sin = _rope_tables()
    m0, m1 = _diag_masks()
    consts = {
        "cosr": np.ascontiguousarray(cos.T).astype(BF16),
        "sinr": np.ascontiguousarray(sin.T).astype(BF16),
        "mk0": np.ascontiguousarray(m0).astype(BF16),
        "mk1": np.ascontiguousarray(m1).astype(BF16),
        "onm": np.ones((P, P), np.float32).astype(BF16),
        "ident": np.eye(P, dtype=np.float32).astype(BF16),
    }
    # rotate_half as a matmul: rot = M @ q (in [d, s] layout); pass M.T as lhsT
    M = np.zeros((P, P), np.float32)
    half = D // 2
    M[np.arange(half), np.arange(half) + half] = -1.0
    M[np.arange(half) + half, np.arange(half)] = 1.0
    consts["protT"] = np.ascontiguousarray(M.T).astype(BF16)
    return consts


def _build_bass():
    import concourse.mybir as mybir
    import concourse.tile as tile
    from concourse import bacc

    f32 = mybir.dt.float32
    bf16 = mybir.dt.bfloat16
    Exp = mybir.ActivationFunctionType.Exp

    nc = bacc.Bacc(None, target_bir_lowering=False, name="mha_tp8")
    # x pre-tiled on host to [sc, g, p, i, f] (4 eo-tiles per 512KB DMA) so
    # every transfer is large and fully contiguous (DMA issue cost on the
    # sync queue engine is ~600ns per dma_start regardless of size)
    xTt = nc.dram_tensor("xTt", [NSC, 4, P, 4, FD], bf16, kind="ExternalInput")
    wqkvT = nc.dram_tensor("wqkvT", [4, P, 4, FLOC], bf16, kind="ExternalInput")
    wdT = nc.dram_tensor("wdT", [2 * P, S], bf16, kind="ExternalInput")
    cosr = nc.dram_tensor("cosr", [P, S], bf16, kind="ExternalInput")
    sinr = nc.dram_tensor("sinr", [P, S], bf16, kind="ExternalInput")
    mk0d = nc.dram_tensor("mk0", [P, 2 * FD], bf16, kind="ExternalInput")
    mk1d = nc.dram_tensor("mk1", [P, 2 * FD], bf16, kind="ExternalInput")
    protT = nc.dram_tensor("protT", [P, P], bf16, kind="ExternalInput")
    ident = nc.dram_tensor("ident", [P, P], bf16, kind="ExternalInput")
    onmd = nc.dram_tensor("onm", [P, P], bf16, kind="ExternalInput")
    # output tiled [c, st, ep, p, k, f] (eo-pairs per 256KB store); host
    # reassembles to [s, e]
    out = nc.dram_tensor("out", [NSC, 4, 2, P, 2, FD], bf16, kind="ExternalOutput")

    with tile.TileContext(nc) as tc:
        with tc.tile_pool(name="const", bufs=1) as const:
            w_sb = const.tile([P, NE, FLOC], bf16, name="w_sb")
            pr = const.tile([P, P], bf16, name="pr")
            idn = const.tile([P, P], bf16, name="idn")
            onm = const.tile([P, P], bf16, name="onm")

            cq = const.tile([P, S], bf16, name="cq")
            sq_t = const.tile([P, S], bf16, name="sq_t")
            mk = [
                const.tile([P, 2 * FD], bf16, name="mk0"),
                const.tile([P, 2 * FD], bf16, name="mk1"),
            ]
            wd_sb = const.tile([P, 2, S], bf16, name="wd_sb")

            qr = const.tile([P, 2, S], bf16, name="qr")
            kr = const.tile([P, S], bf16, name="kr")
            vT = const.tile([P, S], bf16, name="vT")
            vn = const.tile([P, NST, P], bf16, name="vn")

            # ---- Phase A: fused QKV projection + RoPE + v transpose ----
            with tc.tile_pool(name="xs_p", bufs=6) as xpool, \
                 tc.tile_pool(name="ps_qkv", bufs=1, space="PSUM") as pqkv, \
                 tc.tile_pool(name="ps_rot", bufs=2, space="PSUM") as prot_p, \
                 tc.tile_pool(name="ps_vt", bufs=2, space="PSUM") as pvt, \
                 tc.tile_pool(name="rtmp", bufs=3) as rtmp:
                # chunk 0 streams over BOTH HWDGE rings (sync + scalar) in
                # parallel -- the scalar engine has no evac work until ~t=14us
                # so its queue is free early; the first eo-slice of each ring
                # lands alone so the first matmuls start early (all 8 cores
                # contend for HBM here). Later chunks ride the sync ring.
                for sc in range(NSC):
                    ssl = slice(sc * FD, (sc + 1) * FD)
                    psums = [
                        pqkv.tile([P, FD], f32, tag=f"qkv{f}", name=f"ps_qkv{f}_{sc}")
                        for f in range(4)
                    ]
                    for g in range(4):
                        xs = xpool.tile(
                            [P, 4, FD], bf16, tag="xs", name=f"xs_{sc}_{g}"
                        )
                        if sc == 0:
                            ring = nc.sync if g % 2 == 0 else nc.scalar
                            if g < 2:
                                ring.dma_start(w_sb[:, 4 * g, :], wqkvT[g][:, 0, :])
                                ring.dma_start(xs[:, 0, :], xTt[sc, g][:, 0, :])
                                ring.dma_start(
                                    w_sb[:, 4 * g + 1:4 * g + 4, :],
                                    wqkvT[g][:, 1:, :],
                                )
                                ring.dma_start(xs[:, 1:, :], xTt[sc, g][:, 1:, :])
                            else:
                                ring.dma_start(w_sb[:, 4 * g:4 * g + 4, :], wqkvT[g])
                                ring.dma_start(xs, xTt[sc, g])
                            if g == 3:
                                # tables/masks/dense weights follow chunk 0 on
                                # the scalar ring; tiny consts on sync
                                nc.scalar.dma_start(cq, cosr[:, :])
                                nc.scalar.dma_start(sq_t, sinr[:, :])
                                nc.scalar.dma_start(mk[0], mk0d[:, :])
                                nc.scalar.dma_start(mk[1], mk1d[:, :])
                                nc.scalar.dma_start(
                                    wd_sb, wdT.rearrange("(h p) e -> p h e", p=P)
                                )
                                nc.sync.dma_start(pr, protT[:, :])
                                nc.sync.dma_start(idn, ident[:, :])
                                nc.sync.dma_start(onm, onmd[:, :])
                        else:
                            nc.sync.dma_start(xs, xTt[sc, g])
                        for i in range(4):
                            eo = 4 * g + i
                            for f in range(4):
                                nc.tensor.matmul(
                                    psums[f],
                                    lhsT=w_sb[:, eo, f * P:(f + 1) * P],
                                    rhs=xs[:, i, :],
                                    start=(eo == 0),
                                    stop=(eo == NE - 1),
                                )
                    # psum-freeing copies on ScalarE (idle in this phase)
                    for f in range(4):
                        pt = psums[f]
                        if f == 3:
                            nc.scalar.copy(vT[:, ssl], pt)
                            continue
                        dst = qr[:, f, ssl] if f < 2 else kr[:, ssl]
                        qt = rtmp.tile([P, FD], bf16, tag="qt", name=f"qt_{sc}_{f}")
                        nc.scalar.copy(qt, pt)
                        rp = prot_p.tile([P, FD], f32, tag="rot", name=f"rot_{sc}_{f}")
                        nc.tensor.matmul(rp, lhsT=pr, rhs=qt, start=True, stop=True)
                        tt = rtmp.tile([P, FD], bf16, tag="tt", name=f"tt_{sc}_{f}")
                        nc.vector.tensor_mul(tt, rp, sq_t[:, ssl])
                        nc.vector.tensor_mul(dst, qt, cq[:, ssl])
                        nc.vector.tensor_add(dst, dst, tt)
                    # v transpose: 4 PE transposes packed into one PSUM bank,
                    # one ScalarE copy evacuates all four
                    vp = pvt.tile([P, 4 * P], bf16, tag="vt", name=f"vt_{sc}")
                    for k in range(4):
                        j = 4 * sc + k
                        nc.tensor.transpose(
                            vp[:, k * P:(k + 1) * P], vT[:, j * P:(j + 1) * P], idn
                        )
                    nc.scalar.copy(vn[:, 4 * sc:4 * sc + 4, :], vp)

            # ---- Phase B: attention + dense, per 512-query chunk ----
            # Emission order interleaves dense(c) after attention(c+1) so the
            # PE always has independent work during each chunk's softmax tail.
            with tc.tile_pool(name="ps_s", bufs=2, space="PSUM") as ps_s, \
                 tc.tile_pool(name="ps_ctx", bufs=2, space="PSUM") as ps_ctx, \
                 tc.tile_pool(name="ps_rp", bufs=1, space="PSUM") as ps_rp, \
                 tc.tile_pool(name="ps_o", bufs=3, space="PSUM") as ps_o, \
                 tc.tile_pool(name="pt_p", bufs=3) as ptp, \
                 tc.tile_pool(name="acc_p", bufs=3) as accp, \
                 tc.tile_pool(name="rec_p", bufs=2) as recp, \
                 tc.tile_pool(name="ctx_p", bufs=2) as ctxp, \
                 tc.tile_pool(name="out_p", bufs=4) as outp:
                all_csb = {}
                # dense work is emitted as "filler" units (one eo-pair: 4
                # matmuls + 2 evacs + 1 store) sprinkled into the NEXT
                # chunk's attention stream, so dense matmuls cover the PE
                # bubbles left by exp/acc/recip dependency chains
                fillers = []

                def pop_filler(n=1):
                    for _ in range(n):
                        if fillers:
                            fillers.pop(0)()

                def emit_attn(c):
                    qbase = c * FD
                    npairs = 2 * c + 2
                    ctxps, accs = {}, {}
                    for h in range(2):
                        ctxps[h] = ps_ctx.tile(
                            [P, FD], f32, tag="ctx", name=f"ctx_{c}_{h}"
                        )
                        acc = accp.tile(
                            [P, 2 * FD], bf16, tag="acc", name=f"acc_{c}_{h}"
                        )
                        accs[h] = acc
                        qsl = qr[:, h, qbase:qbase + FD]
                        for pi in range(npairs):
                            pt = ptp.tile(
                                [P, 2 * FD], bf16, tag="pt", name=f"pt_{c}_{h}_{pi}"
                            )
                            for half in range(2):
                                j = 2 * pi + half
                                sp = ps_s.tile(
                                    [P, FD], f32, tag="s", name=f"s_{c}_{h}_{j}"
                                )
                                nc.tensor.matmul(
                                    sp,
                                    lhsT=kr[:, j * P:(j + 1) * P],
                                    rhs=qsl, start=True, stop=True,
                                )
                                nc.scalar.activation(
                                    pt[:, half * FD:(half + 1) * FD], sp, Exp
                                )
                            if pi >= npairs - 2:
                                # diagonal pair: zero the causally invisible
                                # region (tiles were computed full-width)
                                nc.vector.tensor_mul(
                                    pt, pt, mk[pi - (npairs - 2)]
                                )
                            if pi == 0:
                                nc.vector.tensor_copy(acc, pt)
                            else:
                                nc.vector.tensor_add(acc, acc, pt)
                            nc.tensor.matmul(
                                ctxps[h],
                                lhsT=vn[:, 2 * pi, :],
                                rhs=pt[:, :FD],
                                start=(pi == 0), stop=False,
                            )
                            nc.tensor.matmul(
                                ctxps[h],
                                lhsT=vn[:, 2 * pi + 1, :],
                                rhs=pt[:, FD:],
                                start=False, stop=(pi == npairs - 1),
                            )

                    # softmax tails after both heads' tile loops; the ones
                    # matmul replicates the denominator across all partitions.
                    # Dense fillers are sandwiched between the two heads'
                    # tails so the PE has work while recip/ct drain on DVE
                    # and the rp bank frees up.
                    for h in range(2):
                        acc = accs[h]
                        rpp = ps_rp.tile([P, FD], f32, tag="rp", name=f"rp_{c}_{h}")
                        nc.tensor.matmul(rpp, lhsT=onm, rhs=acc[:, :FD],
                                         start=True, stop=False)
                        nc.tensor.matmul(rpp, lhsT=onm, rhs=acc[:, FD:],
                                         start=False, stop=True)
                        rec = recp.tile([P, FD], f32, tag="rec", name=f"rec_{c}_{h}")
                        nc.vector.reciprocal_approx_fast(rec, rpp)
                        ct = ctxp.tile([P, FD], bf16, tag=f"ctx{h}", name=f"csb_{c}_{h}")
                        nc.vector.tensor_mul(ct, ctxps[h], rec)
                        all_csb[(c, h)] = ct

                def queue_dense(c):
                    def unit(st, ep):
                        def emit():
                            ot = outp.tile(
                                [P, 2, FD], bf16, tag="ot", name=f"ot_{c}_{st}_{ep}"
                            )
                            for k in range(2):
                                eo = 2 * ep + k
                                op = ps_o.tile(
                                    [P, FD], f32, tag="o", name=f"o_{c}_{st}_{eo}"
                                )
                                for h in range(2):
                                    nc.tensor.matmul(
                                        op,
                                        lhsT=all_csb[(c, h)][:, st * P:(st + 1) * P],
                                        rhs=wd_sb[:, h, eo * FD:(eo + 1) * FD],
                                        start=(h == 0), stop=(h == 1),
                                    )
                                if (st + eo) % 2:
                                    nc.scalar.copy(ot[:, k, :], op)
                                else:
                                    nc.vector.tensor_copy(ot[:, k, :], op)
                            nc.sync.dma_start(out[c, st, ep], ot)
                        return emit
                    for st in range(4):
                        for ep in range(2):
                            fillers.append(unit(st, ep))

                # dense(c) runs as a block after attn(c+1), giving the PE
                # independent work during each chunk's softmax tail
                emit_attn(0)
                emit_attn(1)
                queue_dense(0)
                pop_filler(len(fillers))
                emit_attn(2)
                queue_dense(1)
                pop_filler(len(fillers))
                emit_attn(3)
                queue_dense(2)
                pop_filler(len(fillers))
                queue_dense(3)
                pop_filler(len(fillers))
    nc.compile()
    return nc


def make_in_maps(x, w_qkv, w_dense):
    x = np.asarray(x, np.float32).reshape(S, E)
    w_qkv = np.asarray(w_qkv, np.float32)
    w_dense = np.asarray(w_dense, np.float32)
    # x^T tiled to [sc, g, p, i, f] (4 eo-tiles per DMA) so device DMAs are
    # large and contiguous
    xTt = np.ascontiguousarray(
        x.T.reshape(4, 4, P, NSC, FD).transpose(3, 0, 2, 1, 4)
    ).astype(BF16)
    consts = _host_constants()
    in_maps = []
    scale = np.float32(1.0 / np.sqrt(D))
    for d in range(NCORES):
        g = d // 2
        wq = w_qkv[2 * d * P:(2 * d + 2) * P] * scale
        wk = w_qkv[H * D + g * P: H * D + (g + 1) * P]
        wv = w_qkv[H * D + KVH * D + g * P: H * D + KVH * D + (g + 1) * P]
        wqkvT_d = np.ascontiguousarray(
            np.concatenate([wq, wk, wv], 0).T.reshape(4, 4, P, FLOC)
            .transpose(0, 2, 1, 3)
        ).astype(BF16)
        wdT_d = np.ascontiguousarray(
            w_dense[:, 2 * d * P:(2 * d + 2) * P].T
        ).astype(BF16)
        m = {"xTt": xTt, "wqkvT": wqkvT_d, "wdT": wdT_d}
        m.update(consts)
        in_maps.append(m)
    return in_maps


def kernel(x, w_qkv, w_dense):
    global LAST_RESULT, _BASS_CACHE
    from concourse.bass_utils import run_bass_kernel_spmd

    in_maps = make_in_maps(x, w_qkv, w_dense)
    if _BASS_CACHE is None:
        _BASS_CACHE = _build_bass()
    res = run_bass_kernel_spmd(_BASS_CACHE, in_maps, core_ids=list(range(NCORES)))
    LAST_RESULT = res
    # sum partials over cores, then untile [c, st, ep, p, k, f] -> [s, e]
    acc = np.zeros((NSC, 4, 2, P, 2, FD), np.float32)
    for r in res.results:
        acc += np.asarray(r["out"], dtype=np.float32)
    # [c, st, ep, p, k, f]: s = (c, st, p), e = (ep, k, f)
    full = acc.transpose(0, 1, 3, 2, 4, 5).reshape(S, E)
    return np.ascontiguousarray(full).reshape(B, S, E)


# revision 33
# speedup vs baseline: 1.0313x; 1.0123x over previous
"""Tensor-parallel MultiHeadAttention (QKV + RoPE + GQA causal SDPA + dense)
for 8 Trainium2 NeuronCores.

Sharding (TP as in TPMultiHeadAttention): core d owns query heads {2d, 2d+1}
and the single kv head d//2 (kv heads replicated across core pairs), plus the
matching 256 columns of the dense projection. Each core produces a full-shape
partial output; the all-reduce is a host-side sum over the 8 partials.

Full bf16 pipeline: the PE streams bf16 at the same 1 col/cycle as float32r
(this problem's previous format), but DMA bytes, DVE element ops, and
LDWEIGHTS (FWL) all halve; rel-err vs the fp32 reference is ~7e-3.

Per-core device pipeline:
  1. qkv^T = W_shard @ x^T            -> [f=512, s=2048] (f on partitions)
  2. RoPE on q,k via a permutation matmul (rotate_half) + DVE combine;
     softmax scale folded into the q-side weights
  3. Attention per 512-query chunk, with score k-tiles processed in PAIRS:
     two N=512 score matmuls write bf16 into one shared PSUM bank
     ([128,1024] bf16), one ScalarE exp evacuates the pair, diagonal pairs
     are masked multiplicatively with a host mask constant (diagonal tiles
     are computed full-width so the pair layout stays uniform), the pair is
     added into a bf16 running accumulator (softmax denominator), and two
     v-matmuls accumulate ctx^T in fp32 PSUM.
  4. Denominators: rp[128,512] = allones128.T @ acc via two PE matmuls
     (the sum lands REPLICATED across all 128 partitions - no gpsimd
     broadcast), reciprocal'd on DVE, multiplied into ctx^T.
  5. out[s, e] += ctx^T.T @ wd^T  (accumulate over the 2 local heads),
     written out as bf16; host sums the 8 partials in fp32.
"""

import numpy as np
import ml_dtypes

BF16 = ml_dtypes.bfloat16

B, S, E = 1, 2048, 2048
H, KVH, D = 16, 4, 128
NCORES = 8
P = 128
FD = 512            # matmul moving free dim == one fp32 PSUM bank
NE = E // P         # 16 contraction tiles over the embedding dim
NSC = S // FD       # 4 sequence chunks
NST = S // P        # 16 sequence tiles
FLOC = 4 * P        # local fused qkv rows per core (2 q heads + k + v)
ROPE_BASE = 10000.0

LAST_RESULT = None
_BASS_CACHE = None


def _rope_tables():
    inv = 1.0 / (ROPE_BASE ** (np.arange(0, D, 2, dtype=np.float64) / D))
    t = np.arange(S, dtype=np.float64)
    freqs = np.outer(t, inv)
    emb = np.concatenate([freqs, freqs], axis=-1)  # [S, D]
    return np.cos(emb), np.sin(emb)


def _diag_masks():
    # mask for diagonal pair g (tiles o=2g, 2g+1), laid out [P, 2*FD]:
    # tile o occupies cols [512*(o%2), 512*(o%2)+512); element (r, q) of
    # tile o is visible iff q >= 128*o + r  (q, r local to the 512-chunk)
    q = np.arange(FD)[None, :]
    r = np.arange(P)[:, None]
    masks = []
    for g in range(2):
        cols = []
        for o in (2 * g, 2 * g + 1):
            cols.append((q >= 128 * o + r).astype(np.float32))
        masks.append(np.concatenate(cols, axis=1))
    return masks  # two [P, 1024] arrays


def _host_constants():
    cos, sin = _rope_tables()
    m0, m1 = _diag_masks()
    consts = {
        "cosr": np.ascontiguousarray(cos.T).astype(BF16),
        "sinr": np.ascontiguousarray(sin.T).astype(BF16),
        "mk0": np.ascontiguousarray(m0).astype(BF16),
        "mk1": np.ascontiguousarray(m1).astype(BF16),
        "onm": np.ones((P, P), np.float32).astype(BF16),
        "ident": np.eye(P, dtype=np.float32).astype(BF16),
    }
    # rotate_half as a matmul: rot = M @ q (in [d, s] layout); pass M.T as lhsT
    M = np.zeros((P, P), np.float32)
    half = D // 2
    M[np.arange(half), np.arange(half) + half] = -1.0
    M[np.arange(half) + half, np.arange(half)] = 1.0
    consts["protT"] = np.ascontiguousarray(M.T).astype(BF16)
    return consts


def _build_bass():
    import concourse.mybir as mybir
    import concourse.tile as tile
    from concourse import bacc

    f32 = mybir.dt.float32
    bf16 = mybir.dt.bfloat16
    Exp = mybir.ActivationFunctionType.Exp

    nc = bacc.Bacc(None, target_bir_lowering=False, name="mha_tp8")
    # x pre-tiled on host to [sc, g, p, i, f] (4 eo-tiles per 512KB DMA) so
    # every transfer is large and fully contiguous (DMA issue cost on the
    # sync queue engine is ~600ns per dma_start regardless of size)
    xTt = nc.dram_tensor("xTt", [NSC, 4, P, 4, FD], bf16, kind="ExternalInput")
    wqkvT = nc.dram_tensor("wqkvT", [4, P, 4, FLOC], bf16, kind="ExternalInput")
    wdT = nc.dram_tensor("wdT", [2 * P, S], bf16, kind="ExternalInput")
    cosr = nc.dram_tensor("cosr", [P, S], bf16, kind="ExternalInput")
    sinr = nc.dram_tensor("sinr", [P, S], bf16, kind="ExternalInput")
    mk0d = nc.dram_tensor("mk0", [P, 2 * FD], bf16, kind="ExternalInput")
    mk1d = nc.dram_tensor("mk1", [P, 2 * FD], bf16, kind="ExternalInput")
    protT = nc.dram_tensor("protT", [P, P], bf16, kind="ExternalInput")
    ident = nc.dram_tensor("ident", [P, P], bf16, kind="ExternalInput")
    onmd = nc.dram_tensor("onm", [P, P], bf16, kind="ExternalInput")
    # output tiled [c, st, ep, p, k, f] (eo-pairs per 256KB store); host
    # reassembles to [s, e]
    out = nc.dram_tensor("out", [NSC, 4, 2, P, 2, FD], bf16, kind="ExternalOutput")

    with tile.TileContext(nc) as tc:
        with tc.tile_pool(name="const", bufs=1) as const:
            w_sb = const.tile([P, NE, FLOC], bf16, name="w_sb")
            pr = const.tile([P, P], bf16, name="pr")
            idn = const.tile([P, P], bf16, name="idn")
            onm = const.tile([P, P], bf16, name="onm")

            cq = const.tile([P, S], bf16, name="cq")
            sq_t = const.tile([P, S], bf16, name="sq_t")
            mk = [
                const.tile([P, 2 * FD], bf16, name="mk0"),
                const.tile([P, 2 * FD], bf16, name="mk1"),
            ]
            wd_sb = const.tile([P, 2, S], bf16, name="wd_sb")

            qr = const.tile([P, 2, S], bf16, name="qr")
            kr = const.tile([P, S], bf16, name="kr")
            vT = const.tile([P, S], bf16, name="vT")
            vn = const.tile([P, NST, P], bf16, name="vn")

            # ---- Phase A: fused QKV projection + RoPE + v transpose ----
            with tc.tile_pool(name="xs_p", bufs=6) as xpool, \
                 tc.tile_pool(name="ps_qkv", bufs=1, space="PSUM") as pqkv, \
                 tc.tile_pool(name="ps_rot", bufs=2, space="PSUM") as prot_p, \
                 tc.tile_pool(name="ps_vt", bufs=2, space="PSUM") as pvt, \
                 tc.tile_pool(name="rtmp", bufs=3) as rtmp:
                # chunk 0 streams over BOTH HWDGE rings (sync + scalar) in
                # parallel -- the scalar engine has no evac work until ~t=14us
                # so its queue is free early; the first eo-slice of each ring
                # lands alone so the first matmuls start early (all 8 cores
                # contend for HBM here). Later chunks ride the sync ring.
                for sc in range(NSC):
                    ssl = slice(sc * FD, (sc + 1) * FD)
                    psums = [
                        pqkv.tile([P, FD], f32, tag=f"qkv{f}", name=f"ps_qkv{f}_{sc}")
                        for f in range(4)
                    ]
                    for g in range(4):
                        xs = xpool.tile(
                            [P, 4, FD], bf16, tag="xs", name=f"xs_{sc}_{g}"
                        )
                        if sc == 0:
                            ring = nc.sync if g % 2 == 0 else nc.scalar
                            if g < 2:
                                ring.dma_start(w_sb[:, 4 * g, :], wqkvT[g][:, 0, :])
                                ring.dma_start(xs[:, 0, :], xTt[sc, g][:, 0, :])
                                ring.dma_start(
                                    w_sb[:, 4 * g + 1:4 * g + 4, :],
                                    wqkvT[g][:, 1:, :],
                                )
                                ring.dma_start(xs[:, 1:, :], xTt[sc, g][:, 1:, :])
                            else:
                                ring.dma_start(w_sb[:, 4 * g:4 * g + 4, :], wqkvT[g])
                                ring.dma_start(xs, xTt[sc, g])
                            if g == 3:
                                # tables/masks/dense weights follow chunk 0 on
                                # the scalar ring; tiny consts on sync
                                nc.scalar.dma_start(cq, cosr[:, :])
                                nc.scalar.dma_start(sq_t, sinr[:, :])
                                nc.scalar.dma_start(mk[0], mk0d[:, :])
                                nc.scalar.dma_start(mk[1], mk1d[:, :])
                                nc.scalar.dma_start(
                                    wd_sb, wdT.rearrange("(h p) e -> p h e", p=P)
                                )
                                nc.sync.dma_start(pr, protT[:, :])
                                nc.sync.dma_start(idn, ident[:, :])
                                nc.sync.dma_start(onm, onmd[:, :])
                        else:
                            nc.sync.dma_start(xs, xTt[sc, g])
                        for i in range(4):
                            eo = 4 * g + i
                            for f in range(4):
                                nc.tensor.matmul(
                                    psums[f],
                                    lhsT=w_sb[:, eo, f * P:(f + 1) * P],
                                    rhs=xs[:, i, :],
                                    start=(eo == 0),
                                    stop=(eo == NE - 1),
                                )
                    # psum-freeing copies on ScalarE (idle in this phase)
                    for f in range(4):
                        pt = psums[f]
                        if f == 3:
                            nc.scalar.copy(vT[:, ssl], pt)
                            continue
                        dst = qr[:, f, ssl] if f < 2 else kr[:, ssl]
                        qt = rtmp.tile([P, FD], bf16, tag="qt", name=f"qt_{sc}_{f}")
                        nc.scalar.copy(qt, pt)
                        rp = prot_p.tile([P, FD], f32, tag="rot", name=f"rot_{sc}_{f}")
                        nc.tensor.matmul(rp, lhsT=pr, rhs=qt, start=True, stop=True)
                        tt = rtmp.tile([P, FD], bf16, tag="tt", name=f"tt_{sc}_{f}")
                        nc.vector.tensor_mul(tt, rp, sq_t[:, ssl])
                        nc.vector.tensor_mul(dst, qt, cq[:, ssl])
                        nc.vector.tensor_add(dst, dst, tt)
                    # v transpose: 4 PE transposes packed into one PSUM bank,
                    # one ScalarE copy evacuates all four
                    vp = pvt.tile([P, 4 * P], bf16, tag="vt", name=f"vt_{sc}")
                    for k in range(4):
                        j = 4 * sc + k
                        nc.tensor.transpose(
                            vp[:, k * P:(k + 1) * P], vT[:, j * P:(j + 1) * P], idn
                        )
                    nc.scalar.copy(vn[:, 4 * sc:4 * sc + 4, :], vp)

            # ---- Phase B: attention + dense, per 512-query chunk ----
            # Emission order interleaves dense(c) after attention(c+1) so the
            # PE always has independent work during each chunk's softmax tail.
            with tc.tile_pool(name="ps_s", bufs=2, space="PSUM") as ps_s, \
                 tc.tile_pool(name="ps_ctx", bufs=2, space="PSUM") as ps_ctx, \
                 tc.tile_pool(name="ps_rp", bufs=1, space="PSUM") as ps_rp, \
                 tc.tile_pool(name="ps_o", bufs=3, space="PSUM") as ps_o, \
                 tc.tile_pool(name="pt_p", bufs=3) as ptp, \
                 tc.tile_pool(name="acc_p", bufs=3) as accp, \
                 tc.tile_pool(name="rec_p", bufs=2) as recp, \
                 tc.tile_pool(name="ctx_p", bufs=2) as ctxp, \
                 tc.tile_pool(name="out_p", bufs=4) as outp:
                all_csb = {}
                # dense work is emitted as "filler" units (one eo-pair: 4
                # matmuls + 2 evacs + 1 store) sprinkled into the NEXT
                # chunk's attention stream, so dense matmuls cover the PE
                # bubbles left by exp/acc/recip dependency chains
                fillers = []

                def pop_filler(n=1):
                    for _ in range(n):
                        if fillers:
                            fillers.pop(0)()

                def emit_attn(c):
                    qbase = c * FD
                    npairs = 2 * c + 2
                    ctxps, accs = {}, {}
                    for h in range(2):
                        ctxps[h] = ps_ctx.tile(
                            [P, FD], f32, tag="ctx", name=f"ctx_{c}_{h}"
                        )
                        acc = accp.tile(
                            [P, 2 * FD], bf16, tag="acc", name=f"acc_{c}_{h}"
                        )
                        accs[h] = acc
                        qsl = qr[:, h, qbase:qbase + FD]
                        for pi in range(npairs):
                            pt = ptp.tile(
                                [P, 2 * FD], bf16, tag="pt", name=f"pt_{c}_{h}_{pi}"
                            )
                            for half in range(2):
                                j = 2 * pi + half
                                sp = ps_s.tile(
                                    [P, FD], f32, tag="s", name=f"s_{c}_{h}_{j}"
                                )
                                nc.tensor.matmul(
                                    sp,
                                    lhsT=kr[:, j * P:(j + 1) * P],
                                    rhs=qsl, start=True, stop=True,
                                )
                                nc.scalar.activation(
                                    pt[:, half * FD:(half + 1) * FD], sp, Exp
                                )
                            if pi >= npairs - 2:
                                # diagonal pair: zero the causally invisible
                                # region (tiles were computed full-width)
                                nc.vector.tensor_mul(
                                    pt, pt, mk[pi - (npairs - 2)]
                                )
                            if pi == 0:
                                nc.vector.tensor_copy(acc, pt)
                            else:
                                nc.vector.tensor_add(acc, acc, pt)
                            nc.tensor.matmul(
                                ctxps[h],
                                lhsT=vn[:, 2 * pi, :],
                                rhs=pt[:, :FD],
                                start=(pi == 0), stop=False,
                            )
                            nc.tensor.matmul(
                                ctxps[h],
                                lhsT=vn[:, 2 * pi + 1, :],
                                rhs=pt[:, FD:],
                                start=False, stop=(pi == npairs - 1),
                            )

                    # softmax tails after both heads' tile loops; the ones
                    # matmul replicates the denominator across all partitions.
                    # Dense fillers are sandwiched between the two heads'
                    # tails so the PE has work while recip/ct drain on DVE
                    # and the rp bank frees up.
                    for h in range(2):
                        acc = accs[h]
                        rpp = ps_rp.tile([P, FD], f32, tag="rp", name=f"rp_{c}_{h}")
                        nc.tensor.matmul(rpp, lhsT=onm, rhs=acc[:, :FD],
                                         start=True, stop=False)
                        nc.tensor.matmul(rpp, lhsT=onm, rhs=acc[:, FD:],
                                         start=False, stop=True)
                        rec = recp.tile([P, FD], f32, tag="rec", name=f"rec_{c}_{h}")
                        nc.vector.reciprocal_approx_fast(rec, rpp)
                        ct = ctxp.tile([P, FD], bf16, tag=f"ctx{h}", name=f"csb_{c}_{h}")
                        nc.vector.tensor_mul(ct, ctxps[h], rec)
                        all_csb[(c, h)] = ct
                        pop_filler(2)

                def queue_dense(c):
                    def unit(st, ep):
                        def emit():
                            ot = outp.tile(
                                [P, 2, FD], bf16, tag="ot", name=f"ot_{c}_{st}_{ep}"
                            )
                            for k in range(2):
                                eo = 2 * ep + k
                                op = ps_o.tile(
                                    [P, FD], f32, tag="o", name=f"o_{c}_{st}_{eo}"
                                )
                                for h in range(2):
                                    nc.tensor.matmul(
                                        op,
                                        lhsT=all_csb[(c, h)][:, st * P:(st + 1) * P],
                                        rhs=wd_sb[:, h, eo * FD:(eo + 1) * FD],
                                        start=(h == 0), stop=(h == 1),
                                    )
                                if (st + eo) % 2:
                                    nc.scalar.copy(ot[:, k, :], op)
                                else:
                                    nc.vector.tensor_copy(ot[:, k, :], op)
                            nc.sync.dma_start(out[c, st, ep], ot)
                        return emit
                    for st in range(4):
                        for ep in range(2):
                            fillers.append(unit(st, ep))

                # dense(c) runs as a block after attn(c+1), except 4 units
                # sandwiched into attn(c+1)'s softmax tails where the PE
                # otherwise stalls on the rp-bank/recip chain
                emit_attn(0)
                emit_attn(1)
                queue_dense(0)
                emit_attn(2)
                pop_filler(len(fillers))
                queue_dense(1)
                emit_attn(3)
                pop_filler(len(fillers))
                queue_dense(2)
                pop_filler(len(fillers))
                queue_dense(3)
                pop_filler(len(fillers))
    nc.compile()
    return nc


def make_in_maps(x, w_qkv, w_dense):
    x = np.asarray(x, np.float32).reshape(S, E)
    w_qkv = np.asarray(w_qkv, np.float32)
    w_dense = np.asarray(w_dense, np.float32)
    # x^T tiled to [sc, g, p, i, f] (4 eo-tiles per DMA) so device DMAs are
    # large and contiguous
    xTt = np.ascontiguousarray(
        x.T.reshape(4, 4, P, NSC, FD).transpose(3, 0, 2, 1, 4)
    ).astype(BF16)
    consts = _host_constants()
    in_maps = []
    scale = np.float32(1.0 / np.sqrt(D))
    for d in range(NCORES):
        g = d // 2
        wq = w_qkv[2 * d * P:(2 * d + 2) * P] * scale
        wk = w_qkv[H * D + g * P: H * D + (g + 1) * P]
        wv = w_qkv[H * D + KVH * D + g * P: H * D + KVH * D + (g + 1) * P]
        wqkvT_d = np.ascontiguousarray(
            np.concatenate([wq, wk, wv], 0).T.reshape(4, 4, P, FLOC)
            .transpose(0, 2, 1, 3)
        ).astype(BF16)
        wdT_d = np.ascontiguousarray(
            w_dense[:, 2 * d * P:(2 * d + 2) * P].T
        ).astype(BF16)
        m = {"xTt": xTt, "wqkvT": wqkvT_d, "wdT": wdT_d}
        m.update(consts)
        in_maps.append(m)
    return in_maps


def kernel(x, w_qkv, w_dense):
    global LAST_RESULT, _BASS_CACHE
    from concourse.bass_utils import run_bass_kernel_spmd

    in_maps = make_in_maps(x, w_qkv, w_dense)
    if _BASS_CACHE is None:
        _BASS_CACHE = _build_bass()
    res = run_bass_kernel_spmd(_BASS_CACHE, in_maps, core_ids=list(range(NCORES)))
    LAST_RESULT = res
    # sum partials over cores, then untile [c, st, ep, p, k, f] -> [s, e]
    acc = np.zeros((NSC, 4, 2, P, 2, FD), np.float32)
    for r in res.results:
        acc += np.asarray(r["out"], dtype=np.float32)
    # [c, st, ep, p, k, f]: s = (c, st, p), e = (ep, k, f)
    full = acc.transpose(0, 1, 3, 2, 4, 5).reshape(S, E)
    return np.ascontiguousarray(full).reshape(B, S, E)
